# revision 17
# baseline (speedup 1.0000x reference)
"""GAT autoencoder (6 GAT layers) on Trainium2, 8 NeuronCores.

Strategy: shard destination nodes (and their incoming edges) across 8 cores.
Each core: dense phase computes xp = h @ W (all N nodes, replicated) into an
HBM row table; per-edge attention scores via GPSIMD ap_gather on a replicated
score vector; segment softmax without max-subtraction (scores are small);
segment sums as PE matmuls against one-hot matrices built on-chip from
host-precomputed index data; AllGather of per-core node shards between layers.
"""

import math
import sys

import numpy as np

if "/opt/trn_rl_repo" not in sys.path:
    sys.path.insert(0, "/opt/trn_rl_repo")

import concourse.bacc as bacc
import concourse.bass as bass
import concourse.mybir as mybir
import concourse.tile as tile

F32 = mybir.dt.float32
I16 = mybir.dt.int16
AF = mybir.ActivationFunctionType
ALU = mybir.AluOpType

XP_STRIDE = 512  # fp32 elems per row of the xp HBM scratch table


def _cdiv(a, b):
    return -(-a // b)


# ---------------------------------------------------------------- host prep


def prep_graph(edge_index, N, n_cores):
    """Index-only preprocessing: shard edges by dst, build tile/slot layout."""
    src0 = np.asarray(edge_index[0], np.int64)
    dst0 = np.asarray(edge_index[1], np.int64)
    loop = np.arange(N, dtype=np.int64)
    src = np.concatenate([src0, loop])
    dst = np.concatenate([dst0, loop])
    NPC = N // n_cores
    NT = _cdiv(NPC, 128)

    order = np.argsort(dst, kind="stable")
    src = src[order]
    dst = dst[order]
    lo = np.searchsorted(dst, np.arange(n_cores) * NPC)
    hi = np.searchsorted(dst, (np.arange(n_cores) + 1) * NPC)

    percore = []
    S_T = 1
    for c in range(n_cores):
        es = src[lo[c] : hi[c]]
        ed = dst[lo[c] : hi[c]]
        dloc = ed - c * NPC
        t = dloc >> 7
        m = dloc & 127
        cnts = np.bincount(t, minlength=NT)
        starts = np.zeros(NT, np.int64)
        starts[1:] = np.cumsum(cnts)[:-1]
        i_in_tile = np.arange(len(ed)) - starts[t]
        percore.append((es, ed, t, m, i_in_tile))
        if len(cnts):
            S_T = max(S_T, _cdiv(int(cnts.max()), 128))

    C_cols = NT * S_T
    S_cols = 8 * _cdiv(C_cols, 8)
    n_b = S_cols // 4
    E_k = n_b * 128

    cores = []
    for c in range(n_cores):
        es, ed, t, m, i_in_tile = percore[c]
        gat = np.full((128, NT * S_T * 8), N, np.int16)
        midx = np.full((128, C_cols), -1, np.int16)
        sci = np.full((128, E_k // 16), N, np.int16)

        p = i_in_tile & 127
        cl = i_in_tile >> 7
        gat[i_in_tile & 15, t * (S_T * 8) + (i_in_tile >> 4)] = es
        cg = t * S_T + cl
        midx[p, cg] = m
        b = cg >> 2
        k = cg & 3
        j = (b << 7) + p
        sci[(k << 4) + (j & 15), j >> 4] = es
        sci[64 + (k << 4) + (j & 15), j >> 4] = ed
        # dma_gather consumes the same index stream on each of the 8 Q7
        # cores' 16-partition blocks -> replicate rows [0:16) to all blocks.
        gat = np.tile(gat[:16], (8, 1))
        cores.append(dict(gat=gat, midx=midx, sci=sci))

    return dict(
        N=N, NPC=NPC, NT=NT, S_T=S_T, C_cols=C_cols, S_cols=S_cols,
        n_b=n_b, E_k=E_k, n_cores=n_cores, cores=cores,
    )


def prep_params(params):
    """Reshape/replicate parameter tensors (no arithmetic)."""
    out = []
    for (W, a_s, a_d, b) in params:
        W = np.asarray(W, np.float32)
        H, din, C = W.shape
        w_cat = np.ascontiguousarray(W.transpose(1, 0, 2).reshape(din, H * C))
        a_cat = np.stack(
            [np.asarray(a_s, np.float32).reshape(-1),
             np.asarray(a_d, np.float32).reshape(-1)]
        )  # [2, HC]
        a_rep = np.ascontiguousarray(
            np.broadcast_to(a_cat.reshape(-1)[None], (128, 2 * H * C))
        ).astype(np.float32)
        b_rep = np.ascontiguousarray(
            np.broadcast_to(np.asarray(b, np.float32)[None], (128, C))
        )
        out.append((w_cat, a_rep, b_rep))
    return out


# ---------------------------------------------------------------- builder


def emit(tc, A, cfg, layer_dims, dbg=False):
    """Emit the full 6-layer GAT program into TileContext `tc`.

    A: dict name -> bass.AP of DRAM tensors.
    """
    nc = tc.nc
    N = cfg["N"]
    Np1 = N + 1
    NPC, NT, S_T = cfg["NPC"], cfg["NT"], cfg["S_T"]
    C_cols, S_cols, n_b, E_k = cfg["C_cols"], cfg["S_cols"], cfg["n_b"], cfg["E_k"]
    n_cores = cfg["n_cores"]
    n_layers = len(layer_dims)

    with (
        tc.tile_pool(name="res", bufs=1) as pres,
        tc.tile_pool(name="dram", bufs=1, space="DRAM") as pdram,
    ):
        # DRAM scratch
        xp_t = pdram.tile([Np1, XP_STRIDE], F32)
        agin_t = pdram.tile([128, NPC], F32)
        ag_space = "Shared" if (n_cores > 4 and n_cores % 2 == 0) else "Local"
        agout_ts = [
            pdram.tile([n_cores * 128 * NPC], F32, addr_space=ag_space,
                       name=f"agout{i}")
            for i in range(len(layer_dims) - 1)
        ]

        # resident SBUF tiles
        ident = pres.tile([128, 128], F32)
        nc.sync.dma_start(ident[:], A["ident"])
        iota = pres.tile([128, 128], I16)
        nc.sync.dma_start(iota[:], A["iota"])
        gat_sb = pres.tile([128, NT * S_T * 8], I16)
        nc.sync.dma_start(gat_sb[:], A["gat"])
        sci_sb = pres.tile([128, E_k // 16], I16)
        nc.sync.dma_start(sci_sb[:], A["sci"])
        midx_sb = pres.tile([128, C_cols], I16)
        nc.sync.dma_start(midx_sb[:], A["midx"])
        w_sb = []
        for l, (din, C, H) in enumerate(layer_dims):
            w = pres.tile([din, H * C], F32)
            nc.sync.dma_start(w[:], A[f"w{l}"])
            w_sb.append(w)
        # zero row of xp table (sentinel node)
        zrow = pres.tile([1, XP_STRIDE], F32)
        nc.vector.memset(zrow[:], 0.0)
        nc.sync.dma_start(xp_t[N : N + 1, :], zrow[:])

        for l, (din, C, H) in enumerate(layer_dims):
            HC = H * C
            relu = l not in (2, n_layers - 1)
            last = l == n_layers - 1

            with tc.tile_pool(name=f"L{l}", bufs=1) as pL:
                pSrep_cm = tc.tile_pool(name=f"sr{l}", bufs=1)
                pSrep = pSrep_cm.__enter__()
                brep = pL.tile([128, C], F32, tag="brep")
                nc.sync.dma_start(brep[:], A[f"brep{l}"])
                expsb = pL.tile([128, S_cols * 16], F32, tag="expsb")
                srep = pSrep.tile([128, Np1], F32, tag="srep")
                nc.vector.memset(srep[:, N : N + 1], 0.0)
                phA = tc.tile_pool(name=f"A{l}", bufs=3)
                pA = phA.__enter__()
                psA_cm = tc.tile_pool(name=f"psA{l}", bufs=2, space="PSUM")
                psA = psA_cm.__enter__()
                # per-layer param prep
                arep = pA.tile([128, 2 * HC], F32, tag="arep")
                nc.sync.dma_start(arep[:], A[f"arep{l}"])

                wprod = pA.tile([din, 2 * HC], F32, tag="wprod")
                nc.vector.tensor_mul(
                    wprod[:].rearrange("p (a f) -> p a f", a=2),
                    w_sb[l][:].unsqueeze(1).broadcast_to([din, 2, HC]),
                    arep[:din, :].rearrange("p (a f) -> p a f", a=2),
                )
                wsd = pA.tile([din, 2 * H], F32, tag="wsd")
                nc.vector.tensor_reduce(
                    wsd[:].rearrange("p (a h) -> p a h", a=2),
                    wprod[:].rearrange("p (a h c) -> p a h c", a=2, h=H),
                    axis=mybir.AxisListType.X,
                    op=ALU.add,
                )
                # wrep columns: [16k+q | k<4] = w_src[:, q]; [64+16k+q] = w_dst[:, q]
                wrep = pA.tile([din, 128], F32, tag="wrep")
                nc.vector.memset(wrep[:], 0.0)
                wsd3 = wsd[:].rearrange("p (a h) -> p a h", a=2)
                wv = wrep[:].rearrange("p (a k q) -> p a k q", a=2, k=4)
                nc.vector.tensor_copy(
                    wv[:, :, :, 0:H],
                    wsd3.unsqueeze(2).broadcast_to([din, 2, 4, H]),
                )

                # chunk list: (n0, cnt) in global node order
                chunks = []
                if l == 0:
                    n0 = 0
                    while n0 < N:
                        chunks.append((n0, min(512, N - n0), None))
                        n0 += 512
                else:
                    Cp = layer_dims[l - 1][1]  # previous layer out dim == din
                    view3 = agout_ts[l - 1][: n_cores * Cp * NPC].rearrange(
                        "(r c i) -> r c i", r=n_cores, c=Cp
                    )
                    for r in range(n_cores):
                        i0 = 0
                        while i0 < NPC:
                            cnt = min(512, NPC - i0)
                            chunks.append((r * NPC + i0, cnt, (r, i0)))
                            i0 += cnt

                for (n0, cnt, ri) in chunks:
                    hTc = pA.tile([din, 512], F32, tag="hTc")
                    if l == 0:
                        j = 0
                        while j < cnt:
                            sub = min(128, cnt - j)
                            xt = pA.tile([128, 128], F32, tag="xt")
                            nc.sync.dma_start(
                                xt[:sub, :din], A["x"][n0 + j : n0 + j + sub, :]
                            )
                            pt = psA.tile([128, 128], F32, tag="pt")
                            nc.tensor.transpose(
                                pt[:din, :sub], xt[:sub, :din], ident[:sub, :sub]
                            )
                            nc.scalar.copy(hTc[:, j : j + sub], pt[:din, :sub])
                            j += sub
                    else:
                        r, i0 = ri
                        nc.sync.dma_start(
                            hTc[:, :cnt], view3[r, :, i0 : i0 + cnt]
                        )
                    psS = psA.tile([128, 512], F32, tag="psS")
                    nc.tensor.matmul(
                        psS[:, :cnt], wrep[:], hTc[:, :cnt], start=True, stop=True
                    )
                    nc.scalar.copy(srep[:, n0 : n0 + cnt], psS[:, :cnt])
                    j = 0
                    while j < cnt:
                        sub = min(128, cnt - j)
                        psX = psA.tile([128, 512], F32, tag="psX")
                        nc.tensor.matmul(
                            psX[:sub, :HC],
                            hTc[:, j : j + sub],
                            w_sb[l][:],
                            start=True,
                            stop=True,
                        )
                        xs = pA.tile([128, 512], F32, tag="xs")
                        nc.scalar.copy(xs[:sub, :HC], psX[:sub, :HC])
                        nc.sync.dma_start(
                            xp_t[n0 + j : n0 + j + sub, :HC], xs[:sub, :HC]
                        )
                        j += sub

                psA_cm.__exit__(None, None, None)
                phA.__exit__(None, None, None)

                # ---- phase B: per-edge exp scores
                pB_cm = tc.tile_pool(name=f"B{l}", bufs=1)
                pB = pB_cm.__enter__()
                psB_cm = tc.tile_pool(name=f"psB{l}", bufs=2, space="PSUM")
                psB = psB_cm.__enter__()
                if True:
                    osrc = pB.tile([128, E_k], F32, tag="osrc")
                    srv = srep[:].unsqueeze(2)
                    nc.gpsimd.ap_gather(
                        osrc[:].unsqueeze(2), srv, sci_sb[:],
                        channels=128, num_elems=Np1, d=1, num_idxs=E_k,
                    )
                    otmp = pB.tile([64, E_k], F32, tag="otmp")
                    nc.sync.dma_start(otmp[:, :], osrc[64:128, :])
                    nc.vector.tensor_add(
                        osrc[0:64, :], osrc[0:64, :], otmp[:, :]
                    )
                    nc.vector.scalar_tensor_tensor(
                        osrc[0:64, :], osrc[0:64, :], 0.2, osrc[0:64, :],
                        op0=ALU.mult, op1=ALU.max,
                    )
                    nc.scalar.activation(osrc[0:64, :], osrc[0:64, :], AF.Exp)
                    if dbg and l == 0:
                        nc.sync.dma_start(A["d_srep"], srep[:])
                        nc.sync.dma_start(A["d_sc"], osrc[:])

                    # transpose [64,128] blocks; 8 blocks per psum bank drain
                    ng = _cdiv(n_b, 8)
                    for g in range(ng):
                        nbb = min(8, n_b - 8 * g)
                        ptB = psB.tile([128, 512], F32, tag="ptB")
                        for bb in range(nbb):
                            b = 8 * g + bb
                            nc.tensor.transpose(
                                ptB[:, 64 * bb : 64 * (bb + 1)],
                                osrc[0:64, 128 * b : 128 * (b + 1)],
                                ident[0:64, 0:64],
                            )
                        nc.scalar.copy(
                            expsb[:, 512 * g : 512 * g + 64 * nbb],
                            ptB[:, : 64 * nbb],
                        )

                if dbg and l == 0:
                    nc.sync.dma_start(A["d_exps"], expsb[:])
                # ---- phase C: gather + segment matmuls per dst tile
                psB_cm.__exit__(None, None, None)
                pB_cm.__exit__(None, None, None)
                pSrep_cm.__exit__(None, None, None)
                if True:
                    with (
                        tc.tile_pool(name=f"C{l}", bufs=2) as pC,
                        tc.tile_pool(name=f"psC{l}", bufs=2, space="PSUM") as psC,
                    ):
                        exp16 = expsb[:].rearrange("p (c q) -> p c q", q=16)
                        for t in range(NT):
                            G = pC.tile([128, S_T * HC], F32, tag="G")
                            g3 = G[:].rearrange("p (s f) -> p s f", s=S_T)
                            # SWDGE ring caps one gather at 1024 descriptors
                            for c0 in range(0, S_T, 8):
                                nsl = min(8, S_T - c0)
                                nc.gpsimd.dma_gather(
                                    g3[:, c0 : c0 + nsl, :],
                                    xp_t[:, :HC],
                                    gat_sb[
                                        :,
                                        t * S_T * 8 + c0 * 8 :
                                        t * S_T * 8 + (c0 + nsl) * 8,
                                    ],
                                    num_idxs=nsl * 128,
                                    num_idxs_reg=nsl * 128,
                                    elem_size=HC,
                                    elem_step=XP_STRIDE,
                                    queue_num=(t * 3 + c0 // 8) % nc.num_swdge_queues,
                                )
                            P0 = pC.tile([128, S_T * 128], F32, tag="P0")
                            nc.vector.tensor_tensor(
                                P0[:].rearrange("p (s m) -> p s m", s=S_T),
                                midx_sb[:, t * S_T : (t + 1) * S_T]
                                .unsqueeze(2)
                                .broadcast_to([128, S_T, 128]),
                                iota[:].unsqueeze(1).broadcast_to([128, S_T, 128]),
                                op=ALU.is_equal,
                            )
                            ex = (
                                exp16[:, t * S_T : (t + 1) * S_T, 0:H]
                                .unsqueeze(3)
                                .broadcast_to([128, S_T, H, C])
                            )
                            if dbg and l == 0 and t == 0:
                                nc.sync.dma_start(A["d_graw"], G[:])
                                nc.sync.dma_start(A["d_p0"], P0[:])
                            g4 = G[:].rearrange(
                                "p (s h f) -> p s h f", s=S_T, h=H
                            )
                            nc.vector.tensor_mul(g4, g4, ex)
                            if dbg and l == 0 and t == 0:
                                nc.sync.dma_start(A["d_g"], G[:])

                            pso = psC.tile([128, HC], F32, tag="pso")
                            psd = psC.tile([128, H], F32, tag="psd")
                            p3 = P0[:].rearrange("p (s m) -> p s m", s=S_T)
                            for s in range(S_T):
                                c16 = (t * S_T + s) * 16
                                st = s == 0
                                sp = s == S_T - 1
                                nc.tensor.matmul(
                                    pso[:], p3[:, s, :], g3[:, s, :],
                                    start=st, stop=sp,
                                )
                                nc.tensor.matmul(
                                    psd[:], p3[:, s, :],
                                    expsb[:, c16 : c16 + H],
                                    start=st, stop=sp,
                                )
                            den = pC.tile([128, H], F32, tag="den")
                            nc.scalar.mul(den[:], psd[:], float(H))
                            if dbg and l == 0 and t == 0:
                                dpso = pC.tile([128, HC], F32, tag="dpso")
                                nc.scalar.copy(dpso[:], pso[:])
                                nc.sync.dma_start(A["d_pso"], dpso[:])
                                nc.sync.dma_start(A["d_den"], den[:])
                            rden = pC.tile([128, H], F32, tag="rden")
                            nc.vector.reciprocal(rden[:], den[:])
                            t1 = pC.tile([128, HC], F32, tag="t1")
                            nc.vector.tensor_mul(
                                t1[:].rearrange("p (h c) -> p h c", h=H),
                                pso[:].rearrange("p (h c) -> p h c", h=H),
                                rden[:].unsqueeze(2).broadcast_to([128, H, C]),
                            )
                            if H > 1:
                                t2 = pC.tile([128, C], F32, tag="t2")
                                nc.vector.tensor_reduce(
                                    t2[:],
                                    t1[:]
                                    .rearrange("p (h c) -> p h c", h=H)
                                    .transpose([0, 2, 1]),
                                    axis=mybir.AxisListType.X,
                                    op=ALU.add,
                                )
                            else:
                                t2 = t1
                            t3 = pC.tile([128, C], F32, tag="t3")
                            nc.vector.tensor_add(t3[:], t2[:, :C], brep[:])
                            if relu:
                                nc.scalar.activation(t3[:], t3[:], AF.Relu)
                            if dbg and l == 0 and t == 0:
                                nc.sync.dma_start(A["d_t3"], t3[:])
                            rows = min(128, NPC - 128 * t)
                            if last:
                                nc.sync.dma_start(
                                    A["out"][128 * t : 128 * t + rows, :],
                                    t3[:rows, :],
                                )
                            else:
                                ph = psC.tile([128, 128], F32, tag="ph")
                                nc.tensor.transpose(
                                    ph[:C, :], t3[:], ident[:]
                                )
                                hp = pC.tile([128, 128], F32, tag="hp")
                                nc.scalar.copy(hp[:C, :rows], ph[:C, :rows])
                                nc.sync.dma_start(
                                    agin_t[:C, 128 * t : 128 * t + rows],
                                    hp[:C, :rows],
                                )
            if not last:
                nc.gpsimd.collective_compute(
                    "AllGather",
                    ALU.bypass,
                    ins=[agin_t[:C, :].opt()],
                    outs=[agout_ts[l][: n_cores * C * NPC].opt()],
                    replica_groups=[list(range(n_cores))],
                )


# ---------------------------------------------------------------- runner


LAYER_DIMS = [(128, 128, 4), (128, 64, 4), (64, 32, 4), (32, 64, 1),
              (64, 128, 1), (128, 128, 1)]


def build_program(cfg, layer_dims, in_dim):
    n_cores = cfg["n_cores"]
    N, NPC, NT, S_T, E_k, C_cols = (
        cfg["N"], cfg["NPC"], cfg["NT"], cfg["S_T"], cfg["E_k"], cfg["C_cols"],
    )
    nc = bacc.Bacc("TRN2", target_bir_lowering=False, num_devices=n_cores, num_swdge_queues=4)
    A = {}
    A["x"] = nc.dram_tensor("x", [N, in_dim], F32, kind="ExternalInput").ap()
    A["ident"] = nc.dram_tensor("ident", [128, 128], F32, kind="ExternalInput").ap()
    A["iota"] = nc.dram_tensor("iota", [128, 128], I16, kind="ExternalInput").ap()
    A["gat"] = nc.dram_tensor("gat", [128, NT * S_T * 8], I16, kind="ExternalInput").ap()
    A["sci"] = nc.dram_tensor("sci", [128, E_k // 16], I16, kind="ExternalInput").ap()
    A["midx"] = nc.dram_tensor("midx", [128, C_cols], I16, kind="ExternalInput").ap()
    for l, (din, C, H) in enumerate(layer_dims):
        A[f"w{l}"] = nc.dram_tensor(f"w{l}", [din, H * C], F32, kind="ExternalInput").ap()
        A[f"arep{l}"] = nc.dram_tensor(f"arep{l}", [128, 2 * H * C], F32, kind="ExternalInput").ap()
        A[f"brep{l}"] = nc.dram_tensor(f"brep{l}", [128, C], F32, kind="ExternalInput").ap()
    out_dim = layer_dims[-1][1]
    A["out"] = nc.dram_tensor("out", [NPC, out_dim], F32, kind="ExternalOutput").ap()

    with tile.TileContext(nc) as tc:
        emit(tc, A, cfg, layer_dims)
    nc.compile()
    return nc


def make_in_maps(x, cfg, pp):
    n_cores = cfg["n_cores"]
    common = {
        "x": np.ascontiguousarray(np.asarray(x, np.float32)),
        "ident": np.eye(128, dtype=np.float32),
        "iota": np.ascontiguousarray(
            np.broadcast_to(np.arange(128, dtype=np.int16), (128, 128))
        ),
    }
    for l, (w_cat, a_rep, b_rep) in enumerate(pp):
        common[f"w{l}"] = w_cat
        common[f"arep{l}"] = a_rep
        common[f"brep{l}"] = b_rep
    in_maps = []
    for c in range(n_cores):
        m = dict(common)
        cd = cfg["cores"][c]
        m["gat"] = cd["gat"]
        m["sci"] = cd["sci"]
        m["midx"] = cd["midx"]
        in_maps.append(m)
    return in_maps


def kernel_ex(x, edge_index, params, trace=False):
    from concourse.bass_utils import run_bass_kernel_spmd

    x = np.asarray(x, np.float32)
    edge_index = np.asarray(edge_index)
    N = x.shape[0]
    n_cores = 8
    cfg = prep_graph(edge_index, N, n_cores)
    pp = prep_params(params)
    nc = build_program(cfg, LAYER_DIMS, x.shape[1])
    in_maps = make_in_maps(x, cfg, pp)
    res = run_bass_kernel_spmd(
        nc, in_maps, core_ids=list(range(n_cores)), trace=trace,
        trace_cores=list(range(n_cores)) if trace else None,
    )
    out = np.concatenate([r["out"] for r in res.results], axis=0)
    return out[:N], res


def kernel(x, edge_index, params):
    return kernel_ex(x, edge_index, params)[0]
